# revision 25
# baseline (speedup 1.0000x reference)
"""HEPOS BART cross-attention Trainium2 kernel (bf16, DVFS-aware rewrite).

Shapes (hardcoded): B=2, Tq=1024, Tk=8192, E=1024, H=16, D=64, stride=16,
m = Tk//stride = 512 keys per head.

Sharding: 8 cores = 2 batches x 4 head-groups (4 heads each).
All DMA'd tensors are bf16. PSUM accumulation fp32.

Key scheduling ideas (v2, vs the gated-wave baseline):
  * TRN2's PE DVFS-ramps 0.65 -> 1.2 -> 2.4 GHz and falls back after idle
    gaps (measured: 216ns for an N=512 bf16 matmul warm, 427ns after a gap).
    So the schedule keeps the PE continuously busy: `junk` matmuls (ident x
    zeros into a scratch PSUM tile) pre-ramp the clock during the DMA head
    and fill the unavoidable DMA-bound waits early in the kernel.
  * Input DMA waves are consumption-ordered and finer-grained at the start
    (first matmul needs only wq pair0 + the first half of hsT tqt0).  Waves
    are serialized with `gate` reads on sync so later waves don't steal
    round-robin DMA bandwidth from the critical path.
  * p2a is split into _mm (KV-proj matmuls) and _tp (PE transposes of the
    V half): the PSUM->SBUF copy between them is hidden behind another
    stage's matmuls instead of stalling the PE.
  * The softmax-denominator ones-column is FIRST in vgp, so Z lands in PSUM
    partition 0 and reciprocal_approx_fast reads it directly (the baseline
    needed a native-copy hop because custom DVE ops misread partition-
    *shifted* PSUM).
  * Tail: po(2), po(3) and the dd0 halves of po(4..7) are held back to
    cover the last head's normalize-chain latency (recip -> partition
    broadcast -> mult); po(6), po(7) run last.
  * Output DMA merged to one 256KB transfer per po tile; out dram layout is
    [8, 128, 1024] (t8, token, E) so the SBUF tile is contiguous.
bk is dropped (constant key shift cancels in softmax); bv folded into the
host-side final bias (bv @ Wo.T + bo).
"""

import numpy as np
import ml_dtypes

import concourse.bass as bass
import concourse.bacc as bacc
import concourse.tile as tile
from concourse import library_config, mybir
from concourse.masks import make_identity

B, Tq, Tk, E, H, D = 2, 1024, 8192, 1024, 16, 64
STRIDE = 16
M = Tk // STRIDE          # 512 keys per head
HPC = 4                   # heads per core
NCORES = 8
F32 = mybir.dt.float32
BF16 = mybir.dt.bfloat16
NPBF16 = ml_dtypes.bfloat16

# junk-matmul fill counts (tuning knobs): warmup, and fills before
# DMA-gated stages.
J0 = 10   # warmup before ph1(0,0)
J2 = 7    # before p2a_mm(0)  (kvg0 wave)
J3 = 2    # before p2a_mm(1)  (kvg1 wave)
J4 = 0    # before p2a_mm(2)  (kvg2 wave)
J5 = 0    # before ph1(0,1)   (hsT tqt1 wave)
J6 = 0    # before p2a_mm(3)  (kvg3 wave)

DIRECT_RECIP = True    # Z lands in PSUM partition 0 (ones col first,
                       # 31-zero-col pad keeps O 32-aligned): unshifted
                       # custom-DVE PSUM reads are OK; shifted ones are not
                       # (verified rel err 5e3 at offset 64).
EOT1_ENGINE = "vector"  # engine for the second half of the po output copy
                        # (GPSIMD cannot read PSUM)


def build_program():
    nc = bacc.Bacc("TRN2", target_bir_lowering=False)

    # dram tensors already in SBUF layout
    # hsT col = tqt*4096 + e*512 + t ; wq col = pair*1024 + e*128 + j
    hsT = nc.dram_tensor("hsT", [128, 8 * Tq], BF16, kind="ExternalInput")
    wq = nc.dram_tensor("wq", [128, 2048], BF16, kind="ExternalInput")
    wkv = nc.dram_tensor("wkv", [128, 4096], BF16, kind="ExternalInput")
    kvg = nc.dram_tensor("kvg", [HPC, 128, 8 * M], BF16, kind="ExternalInput")
    wo = nc.dram_tensor("wo", [128, 2048], BF16, kind="ExternalInput")
    bqp = nc.dram_tensor("bqp", [2, 128, 1], F32, kind="ExternalInput")
    out = nc.dram_tensor("out", [8, 128, 2 * 512], BF16, kind="ExternalOutput")

    with tile.TileContext(nc) as tc:
        with (
            tc.tile_pool(name="consts", bufs=1) as consts,
            tc.tile_pool(name="expp", bufs=3) as expp,
            tc.tile_pool(name="rowp", bufs=3) as rowp,
            tc.tile_pool(name="pop", bufs=4) as pop,
            tc.tile_pool(name="psA", bufs=3, space="PSUM") as psA,
            tc.tile_pool(name="psB", bufs=2, space="PSUM") as psB,
        ):
            # ---- persistent SBUF tiles -------------------------------------
            hsT_sb = consts.tile([128, 8 * Tq], BF16)
            wq_sb = consts.tile([128, 2048], BF16)
            wkv_sb = consts.tile([128, 4096], BF16)
            wo_sb = consts.tile([128, 2048], BF16)
            kvg_sb = [consts.tile([128, 8 * M], BF16, name=f"kvg{h}")
                      for h in range(HPC)]
            kv_sb = [consts.tile([128, M], BF16, name=f"kv{h}")
                     for h in range(HPC)]
            vgp_sb = [consts.tile([128, 4, 128], BF16, name=f"vgp{h}")
                      for h in range(HPC)]
            qt_sb = [consts.tile([D, Tq], BF16, name=f"qt{h}")
                     for h in range(HPC)]
            outT_sb = [consts.tile([128, Tq], BF16, name=f"outT{dd}")
                       for dd in range(2)]
            bq_sb = [consts.tile([128, 1], F32, name=f"bq{p}") for p in range(2)]
            junk_sb = consts.tile([128, 512], BF16)

            # ---- input DMA waves ------------------------------------------
            # Consumption order; gates on sync serialize waves so earlier
            # (critical) bytes get the full round-robin bandwidth.
            gate_sb = consts.tile([1, 8], BF16)
            _gate_i = [0]

            def gate(sl):
                i = _gate_i[0]
                _gate_i[0] += 1
                nc.sync.dma_start(out=gate_sb[:, i:i + 1], in_=sl)

            # ---- warmup constants (no DMA deps) ---------------------------
            identf = consts.tile([128, 128], F32)
            make_identity(nc, identf)
            ident = consts.tile([128, 128], BF16)
            nc.vector.tensor_copy(ident[:], identf[:])
            nc.vector.memset(junk_sb[:], 0.0)
            for h in range(HPC):
                # ones col 0 (-> Z in PSUM partition 0), zeros 1:64 (pad
                # so O occupies partitions 64:128; >32-partition PSUM reads
                # must start at partition 0 or 64), V in 64:128
                nc.vector.memset(vgp_sb[h][:, :, 0:1], 1.0)
                nc.vector.memset(vgp_sb[h][:, :, 1:64], 0.0)


            S = nc.sync.dma_start
            G = nc.gpsimd.dma_start
            # DMA model (measured): two rings (sync, gpsimd) share the 16
            # DMA engines ~50/50 while both have pending work (~179GB/s per
            # ring); within a ring transfers complete strictly in FIFO
            # order.  Layout: each ring carries half of W1 (ph1's deps),
            # then a gate on the OTHER ring's W1 tail, then half of each
            # later wave -- so W1 owns the full bandwidth first and the
            # later waves stream on both rings in consumption order.
            G(out=wq_sb[:, 0:1024], in_=wq[:, 0:1024])
            G(out=hsT_sb[:, 0:1024], in_=hsT[:, 0:1024])
            G(out=hsT_sb[:, 1024:2048], in_=hsT[:, 1024:2048])
            gate_g = consts.tile([1, 1], BF16)
            nc.gpsimd.dma_start(out=gate_g[:], in_=hsT_sb[127:128, 4095:4096])
            G(out=kvg_sb[0][:, 2048:4096], in_=kvg[0][:, 2048:4096])
            G(out=kvg_sb[1][:, 2048:4096], in_=kvg[1][:, 2048:4096])
            G(out=kvg_sb[2][:, 2048:4096], in_=kvg[2][:, 2048:4096])
            G(out=hsT_sb[:, 4096:6144], in_=hsT[:, 4096:6144])
            G(out=kvg_sb[3][:, 2048:4096], in_=kvg[3][:, 2048:4096])
            G(out=wo_sb[:, 0:1024], in_=wo[:, 0:1024])
            S(out=bq_sb[0][:], in_=bqp[0])
            S(out=bq_sb[1][:], in_=bqp[1])
            S(out=wq_sb[:, 1024:2048], in_=wq[:, 1024:2048])
            S(out=hsT_sb[:, 2048:3072], in_=hsT[:, 2048:3072])
            S(out=hsT_sb[:, 3072:4096], in_=hsT[:, 3072:4096])
            gate(hsT_sb[127:128, 2047:2048])
            S(out=kvg_sb[0][:, 0:2048], in_=kvg[0][:, 0:2048])
            S(out=wkv_sb[:, 0:1024], in_=wkv[:, 0:1024])
            S(out=kvg_sb[1][:, 0:2048], in_=kvg[1][:, 0:2048])
            S(out=wkv_sb[:, 1024:2048], in_=wkv[:, 1024:2048])
            S(out=kvg_sb[2][:, 0:2048], in_=kvg[2][:, 0:2048])
            S(out=wkv_sb[:, 2048:3072], in_=wkv[:, 2048:3072])
            S(out=hsT_sb[:, 6144:8192], in_=hsT[:, 6144:8192])
            S(out=kvg_sb[3][:, 0:2048], in_=kvg[3][:, 0:2048])
            S(out=wkv_sb[:, 3072:4096], in_=wkv[:, 3072:4096])
            S(out=wo_sb[:, 1024:2048], in_=wo[:, 1024:2048])

            def junk(n):
                # DVFS keep-alive: PE matmuls with no DMA dependencies.
                for _ in range(n):
                    ps_j = psA.tile([128, 2, 512], F32, tag="A", name="ps_j")
                    nc.tensor.matmul(ps_j[:, 0, :], junk_sb[:, 0:128],
                                     junk_sb[:], start=True, stop=True)

            # ---- stage closures -------------------------------------------
            def ph1_both(tqt):
                # both head-pairs interleaved e-wise: each DMA'd hsT chunk
                # feeds two back-to-back matmuls, so the PE's demand rate
                # matches the W1 stream and DVFS stays up
                ps_qt = psA.tile([128, 2, 512], F32, tag="A", name="ps_qt")
                for e in range(8):
                    for pair in range(2):
                        nc.tensor.matmul(
                            ps_qt[:, pair, :],
                            wq_sb[:, pair * 1024 + e * 128:
                                  pair * 1024 + (e + 1) * 128],
                            hsT_sb[:, tqt * 4096 + e * 512:
                                   tqt * 4096 + (e + 1) * 512],
                            start=(e == 0), stop=(e == 7))
                for pair in range(2):
                    for sub in range(2):
                        h = 2 * pair + sub
                        nc.vector.tensor_scalar_add(
                            qt_sb[h][:, tqt * 512: tqt * 512 + 512],
                            ps_qt[pair * 64 + sub * 64 - pair * 64:
                                  0, 0, :] if False else
                            ps_qt[sub * 64:(sub + 1) * 64, pair, :],
                            bq_sb[pair][sub * 64:(sub + 1) * 64, 0:1])

            _pkv = [None] * HPC

            def p2a_mm(h):
                ps_kv = psA.tile([128, 2, 512], F32, tag="A", name="ps_kv")
                for e in range(8):
                    nc.tensor.matmul(
                        ps_kv[:, 0, :],
                        wkv_sb[:, (h * 8 + e) * 128:(h * 8 + e + 1) * 128],
                        kvg_sb[h][:, e * M:(e + 1) * M],
                        start=(e == 0), stop=(e == 7))
                nc.scalar.copy(kv_sb[h][:], ps_kv[:, 0, :])

            def p2a_tp(h):
                ps_vt = psB.tile([128, 4, D], BF16, tag="B", name="ps_vt")
                for mc in range(4):
                    nc.tensor.transpose(
                        ps_vt[:, mc, :],
                        kv_sb[h][64:128, mc * 128:(mc + 1) * 128],
                        ident[64:128, 64:128])
                nc.vector.tensor_copy(vgp_sb[h][:, :, 64:64 + D], ps_vt[:])

            pending = []

            def flush():
                while pending:
                    pending.pop(0)()

            def sc_part(tqt, h):
                expT = expp.tile([128, 4, 512], BF16, tag="expT", name="expT")
                for mcp in range(2):
                    ps_sc = psA.tile([128, 2, 512], F32, tag="A", name="ps_sc")
                    for sub in range(2):
                        mc = 2 * mcp + sub
                        nc.tensor.matmul(
                            ps_sc[:, sub, :],
                            kv_sb[h][0:D, mc * 128:(mc + 1) * 128],
                            qt_sb[h][:, tqt * 512: tqt * 512 + 512],
                            start=True, stop=True)
                    nc.scalar.activation(
                        expT[:, 2 * mcp:2 * mcp + 2, :],
                        ps_sc[:],
                        mybir.ActivationFunctionType.Exp)
                return expT

            def av_part(tqt, h, expT):
                if pending:
                    pending.pop(0)()
                ps_ov = psB.tile([128, 512], F32, tag="B", name="ps_ov")
                for mc in range(4):
                    nc.tensor.matmul(
                        ps_ov[0:128, :],
                        vgp_sb[h][:, mc, :],
                        expT[:, mc, :],
                        start=(mc == 0), stop=(mc == 3))
                # Z is in PSUM partition 64 (ones column last in vgp).
                # Native InstReciprocal reads shifted PSUM fine (custom DVE
                # ops don't), fusing the baseline's copy+recip pair.
                rinv = rowp.tile([1, 512], F32, tag="rinv", name="rinv")
                if DIRECT_RECIP:
                    # Z is in PSUM partition 0: unshifted custom-DVE read
                    nc.vector.reciprocal_approx_fast(rinv[:], ps_ov[0:1, :])
                else:
                    zrow = rowp.tile([1, 512], F32, tag="zrow", name="zrow")
                    nc.vector.tensor_copy(zrow[:], ps_ov[0:1, :])
                    nc.vector.reciprocal_approx_fast(rinv[:], zrow[:])
                rinv_b = rowp.tile([D, 512], F32, tag="rinv_b", name="rinv_b")
                nc.gpsimd.partition_broadcast(rinv_b[:], rinv[:])

                # defer the normalize-mult one stage so DVE never stalls on
                # the gpsimd broadcast
                def _mult(tqt=tqt, h=h, ps_ov=ps_ov, rinv_b=rinv_b):
                    nc.vector.tensor_tensor(
                        outT_sb[h // 2][(h % 2) * D:(h % 2 + 1) * D,
                                        tqt * 512: tqt * 512 + 512],
                        ps_ov[64:64 + D, :],
                        rinv_b[:],
                        op=mybir.AluOpType.mult)
                pending.append(_mult)

            def _po_out(t8, ps_po):
                po_sb = pop.tile([128, 2, 512], BF16, tag="po", name="po_sb")
                nc.scalar.copy(po_sb[:, 0, :], ps_po[:, 0, :])
                if EOT1_ENGINE == "gpsimd":
                    nc.gpsimd.tensor_copy(po_sb[:, 1, :], ps_po[:, 1, :])
                else:
                    nc.vector.tensor_copy(po_sb[:, 1, :], ps_po[:, 1, :])
                nc.sync.dma_start(out=out[t8], in_=po_sb[:])

            def po(t8):
                ps_po = psA.tile([128, 2, 512], F32, tag="A", name="ps_po")
                for eot in range(2):
                    for dd in range(2):
                        nc.tensor.matmul(
                            ps_po[:, eot, :],
                            outT_sb[dd][:, t8 * 128:(t8 + 1) * 128],
                            wo_sb[:, dd * E + eot * 512:
                                  dd * E + eot * 512 + 512],
                            start=(dd == 0), stop=(dd == 1))
                _po_out(t8, ps_po)

            def po_dd0(t8):
                ps_po = psA.tile([128, 2, 512], F32, tag="A", name="ps_po")
                for eot in range(2):
                    nc.tensor.matmul(
                        ps_po[:, eot, :],
                        outT_sb[0][:, t8 * 128:(t8 + 1) * 128],
                        wo_sb[:, eot * 512: eot * 512 + 512],
                        start=True, stop=False)
                return ps_po

            def po_dd1(t8, ps_po):
                flush()
                for eot in range(2):
                    nc.tensor.matmul(
                        ps_po[:, eot, :],
                        outT_sb[1][:, t8 * 128:(t8 + 1) * 128],
                        wo_sb[:, E + eot * 512: E + eot * 512 + 512],
                        start=False, stop=True)
                _po_out(t8, ps_po)

            # ---- schedule -------------------------------------------------
            junk(J0)
            ph1_both(0)
            junk(J2)
            p2a_mm(0)
            junk(2)          # cover the kv PSUM->SBUF copy latency (head 0)
            p2a_tp(0)
            e00 = sc_part(0, 0)
            junk(J3)
            p2a_mm(1)
            av_part(0, 0, e00)
            p2a_tp(1)
            e01 = sc_part(0, 1)
            junk(J4)
            p2a_mm(2)
            av_part(0, 1, e01)
            p2a_tp(2)
            e02 = sc_part(0, 2)
            junk(J5)
            av_part(0, 2, e02)
            ph1_both(1)
            junk(J6)
            p2a_mm(3)
            e10 = sc_part(1, 0)
            p2a_tp(3)
            e03 = sc_part(0, 3)
            av_part(1, 0, e10)
            e11 = sc_part(1, 1)
            av_part(0, 3, e03)
            e12 = sc_part(1, 2)
            av_part(1, 1, e11)
            e13 = sc_part(1, 3)
            av_part(1, 2, e12)
            po(0)
            av_part(1, 3, e13)
            po(1)
            po(2)
            flush()   # m(1,2)/m(1,3) enter the vector queue AFTER the po
                      # casts, so po(3)/dd0 psA-ring WARs clear quickly and
                      # the mults still finish before po_dd1 needs them
            po(3)
            pp4 = po_dd0(4)
            pp5 = po_dd0(5)
            po_dd1(4, pp4)
            po_dd1(5, pp5)
            po(6)
            po(7)

    nc.compile()
    return nc


_NC = None


def _get_nc():
    global _NC
    if _NC is None:
        _NC = build_program()
    return _NC


def shard_inputs(hidden_states, key_value_states, Wq, bq, Wk, bk, Wv, bv, Wo,
                 bo, stride):
    stride = int(stride)
    assert stride == STRIDE
    scale = float(D) ** -0.5
    bf = lambda a: np.ascontiguousarray(a).astype(NPBF16)
    in_maps = []
    for c in range(NCORES):
        b, g = divmod(c, 4)
        h0 = g * HPC
        r0, r1 = h0 * D, (h0 + HPC) * D
        # hsT [128, 8192]: col tqt*4096+e*512+t = hs[b][tqt*512+t, e*128+p]
        hsT_c = hidden_states[b].T.reshape(8, 128, 2, 512) \
            .transpose(1, 2, 0, 3).reshape(128, 8 * Tq)
        # wq [128, 2048]: col pair*1024+e*128+j = Wq_s[pair*128+j, e*128+p]
        Wqs = (Wq[r0:r1, :] * scale)
        wq_c = Wqs.T.reshape(8, 128, 2, 128).transpose(1, 2, 0, 3) \
            .reshape(128, 2048)
        # wkv [128, 4096]: col (h*8+e)*128+j: j<64 Wk, j>=64 Wv
        K3 = Wk[r0:r1, :].reshape(HPC, D, E)
        V3 = Wv[r0:r1, :].reshape(HPC, D, E)
        C = np.concatenate([K3, V3], axis=1)          # [4, 128, 1024]
        wkv_c = C.reshape(4, 128, 8, 128).transpose(3, 0, 2, 1) \
            .reshape(128, 4096)
        # kvg [4, 128, 4096]: col e*512+m = kv[b, h0+h+16m, e*128+p]
        kvg_c = np.empty((HPC, 128, 8 * M), NPBF16)
        for hl in range(HPC):
            R = key_value_states[b, (h0 + hl)::STRIDE, :]     # [512, 1024]
            kvg_c[hl] = bf(R.T.reshape(8, 128, M).transpose(1, 0, 2)
                           .reshape(128, 8 * M))
        # wo [128, 2048]: col dd*1024+n = Wo[n, r0+dd*128+p]
        wo_c = Wo[:, r0:r1].T.reshape(2, 128, E).transpose(1, 0, 2) \
            .reshape(128, 2048)
        bqp_c = (bq[r0:r1] * scale).astype(np.float32).reshape(2, 128, 1)
        in_maps.append({
            "hsT": bf(hsT_c),
            "wq": bf(wq_c),
            "wkv": bf(wkv_c),
            "kvg": np.ascontiguousarray(kvg_c),
            "wo": bf(wo_c),
            "bqp": bqp_c,
        })
    return in_maps


def combine_outputs(results, Wv, bv, Wo, bo):
    final_bias = (bv @ Wo.T + bo).astype(np.float32)  # [E]
    out = np.zeros((B, Tq, E), np.float32)
    for c in range(NCORES):
        b = c // 4
        # out dram [8, 128, 1024]: token = t8*128 + p, col = eot*512 + x
        o = results[c]["out"].astype(np.float32).reshape(Tq, E)
        out[b] += o
    out += final_bias[None, None, :]
    return out


def kernel(hidden_states, key_value_states, Wq, bq, Wk, bk, Wv, bv, Wo, bo,
           stride, _trace=False, _trace_kwargs=None):
    from concourse.bass_utils import run_bass_kernel_spmd

    args = [np.asarray(x, np.float32) for x in
            (hidden_states, key_value_states, Wq, bq, Wk, bk, Wv, bv, Wo, bo)]
    (hidden_states, key_value_states, Wq, bq, Wk, bk, Wv, bv, Wo, bo) = args
    in_maps = shard_inputs(hidden_states, key_value_states, Wq, bq, Wk, bk,
                           Wv, bv, Wo, bo, stride)
    nc = _get_nc()
    res = run_bass_kernel_spmd(
        nc, in_maps, list(range(NCORES)),
        trace=_trace, **(_trace_kwargs or {}))
    out = combine_outputs(res.results, Wv, bv, Wo, bo)
    kernel.last_run = res
    return out
